# revision 13
# baseline (speedup 1.0000x reference)
"""Per-sample ResNet block (conv3x3 -> relu -> conv3x3 -> +x -> relu) on 8 trn2 cores.

Full inputs: x [16,256,64,64] f32, kernel1/kernel2 [16,256,256,3,3] f32.
Sharding: pure data parallelism, 2 samples per core.

Per-core bass/tile kernel:
  - x sample is stored in SBUF as two 128-channel chunks, zero-padded to 66x66
    so each conv tap (dy,dx) is a shifted AP slice.
  - conv = sum over (ci_chunk, tap) of matmul(lhsT=kT[ci,co], rhs=x_shift[ci,n])
    accumulated in PSUM over 18 matmuls per [128 co x 512 n] tile.
  - weights are DMA'd [co, ci*9] (contiguous) and transposed on the PE
    (out = in.T via identity) to get [ci, co] tiles.
  - residual add is one extra identity matmul into the same PSUM accumulation.
  - relu via ScalarE activation evacuates PSUM -> SBUF.

Modes (default bf16; measured on trn2, 8 cores):
  bf16: bf16 storage/matmuls, fp32 PSUM + exact fp32 residual add on DVE.
        ~287 us HW exec, rel err ~1.8e-3. FWL halves weight-load time, which
        fp32 dtypes cannot use, putting the LDW+MM cadence at the N=512
        streaming roofline.
  f32r: fp32 storage, matmuls in float32r (single-pass PE: 1 cyc/row at
        N>=256). ~326 us, rel err ~2e-4. Producers feeding fp32r matmuls must
        emit fp32r-tagged data (walrus birverifier rule), so x and the
        identity get DVE rounding copies and relu/weight copies write fp32r.
  f32:  plain fp32 matmuls (4 cyc/row). Slow reference fallback.
"""

import numpy as np
from contextlib import ExitStack

import concourse.bass as bass
import concourse.mybir as mybir
import concourse.tile as tile
from concourse import bacc
from concourse.bass_utils import run_bass_kernel_spmd
from concourse.masks import make_identity

N_CORES = 8
B_FULL = 16
BPC = B_FULL // N_CORES  # samples per core
C = 256
H = W = 64
HP = WP = 66  # padded
P = 128
CCH = C // P  # channel chunks: 2
NT = 8        # spatial tiles (rows of 8) per image: 64 rows / 8
TR = 8        # rows per spatial tile
F32 = mybir.dt.float32
F32R = mybir.dt.float32r
BF16 = mybir.dt.bfloat16


def build_nc(mode="f32r"):
    sd = BF16 if mode == "bf16" else F32          # storage dtype
    mmd = {"f32r": F32R, "bf16": BF16, "f32": F32}[mode]  # matmul dtype

    def mm(ap):
        # view a storage AP with the matmul dtype
        return ap.bitcast(mmd) if mmd != sd else ap

    nc = bacc.Bacc("TRN2", target_bir_lowering=False, debug=False)

    x_d = nc.dram_tensor("x", [BPC, C, H, W], F32, kind="ExternalInput")
    k1_d = nc.dram_tensor("kernel1", [BPC, C, C, 3, 3], F32, kind="ExternalInput")
    k2_d = nc.dram_tensor("kernel2", [BPC, C, C, 3, 3], F32, kind="ExternalInput")
    out_d = nc.dram_tensor("out", [BPC, C, H, W], F32, kind="ExternalOutput")

    with tile.TileContext(nc) as tc, ExitStack() as ctx:
        persist = ctx.enter_context(tc.tile_pool(name="persist", bufs=1))
        kraw_p = ctx.enter_context(tc.tile_pool(name="kraw", bufs=2))
        xs_p = ctx.enter_context(tc.tile_pool(name="xs", bufs=2))
        acc_p = ctx.enter_context(tc.tile_pool(name="acc", bufs=4, space="PSUM"))
        tr_p = ctx.enter_context(tc.tile_pool(name="tr", bufs=4, space="PSUM"))
        out_p = ctx.enter_context(tc.tile_pool(name="outs", bufs=4))

        ident = persist.tile([P, P], sd, tag="ident", name="ident")
        make_identity(nc, ident)
        if mmd == F32R:
            ident_r = persist.tile([P, P], F32, tag="ident_r", name="ident_r")
            nc.vector.tensor_copy(ident_r.bitcast(F32R), ident)
            ident = ident_r

        # PE p-state warmup: the tensor engine ramps to full clock only
        # after ~3us of continuous execution. Real weights arrive at ~11us
        # (DMA + cast latency); until then run dummy transposes so the
        # first real matmuls start at full speed instead of ~2x slower.
        if sd == BF16:
            wrhs = persist.tile([P, 512], sd, tag="wrhs", name="wrhs")
            nc.gpsimd.memset(wrhs, 0.0)
            for _ in range(10):
                warm = tr_p.tile([P, 512], F32, tag="tr", name="warm")
                nc.tensor.matmul(warm[:], ident, wrhs[:], start=True,
                                 stop=True)

        # persistent padded images + transposed weights
        xp = [persist.tile([P, CCH, HP, WP], sd, tag=f"xp{i}", name=f"xp{i}")
              for i in range(2)]
        hp = persist.tile([P, CCH, HP, WP], sd, tag="hp", name="hp")
        # bf16 mode: keep an fp32 copy of x resident for the exact residual
        # add on DVE (replaces the identity matmul on PE)
        xf = None
        if sd == BF16:
            xf = [persist.tile([P, CCH, H, W], F32, tag=f"xf{i}", name=f"xf{i}")
                  for i in range(2)]
        k1T = persist.tile([P, CCH, CCH, 9, P], sd, tag="k1T", name="k1T")
        k2T = persist.tile([P, CCH, CCH, 9, P], sd, tag="k2T", name="k2T")

        # zero the 1-px borders of the padded tiles (via DVE copy from a zero
        # row: fp32r can't be memset directly, and the fp32r matmul requires
        # fp32r-tagged producers). Emitted lazily, just before first use.
        zrow = persist.tile([P, WP], sd, tag="zrow", name="zrow")
        nc.vector.memset(zrow, 0.0)

        def zero_borders(t):
            for c in range(CCH):
                if sd == BF16:
                    # gpsimd is idle and has no framework preamble backlog
                    nc.gpsimd.memset(t[:, c, 0, :], 0.0)
                    nc.gpsimd.memset(t[:, c, HP - 1, :], 0.0)
                    nc.gpsimd.memset(t[:, c, 1:HP - 1, 0], 0.0)
                    nc.gpsimd.memset(t[:, c, 1:HP - 1, WP - 1], 0.0)
                else:
                    nc.vector.tensor_copy(mm(t[:, c, 0, :]), zrow[:, :WP])
                    nc.vector.tensor_copy(mm(t[:, c, HP - 1, :]), zrow[:, :WP])
                    nc.vector.tensor_copy(
                        mm(t[:, c, 1:HP - 1, 0]), zrow[:, :HP - 2])
                    nc.vector.tensor_copy(
                        mm(t[:, c, 1:HP - 1, WP - 1]), zrow[:, :HP - 2])

        def load_k_chunk(k_d, b, coc):
            kr = kraw_p.tile([P, C, 9], sd, tag="kr", name="kr")
            src = k_d[b, coc * P:(coc + 1) * P].rearrange(
                "co ci kh kw -> co ci (kh kw)")
            if sd == F32:
                nc.sync.dma_start(out=kr[:], in_=src)
            else:
                # HWDGE f32 DMA to staging, then cast to bf16 on ACT (keeps
                # the startup-critical DVE queue free; SWDGE cast-DMA stalls
                # on Q7 descriptor emission)
                krs = kraw_p.tile([P, C, 9], F32, tag="krs", name="krs")
                nc.sync.dma_start(out=krs[:], in_=src)
                nc.scalar.activation(
                    kr[:], krs[:], mybir.ActivationFunctionType.Copy)
            return kr

        def dma_k(k_d, b, coc, split=False):
            # f32 staging DMA only; cast emitted separately (startup latency)
            krs = kraw_p.tile([P, C, 9], F32, tag="krs", name="krs")
            src = k_d[b, coc * P:(coc + 1) * P].rearrange(
                "co ci kh kw -> co ci (kh kw)")
            if split:
                nc.sync.dma_start(out=krs[:, :P], in_=src[:, :P])
                nc.sync.dma_start(out=krs[:, P:], in_=src[:, P:])
            else:
                nc.sync.dma_start(out=krs[:], in_=src)
            return krs

        def cast_k(krs, kr=None, lo=0, hi=C):
            if kr is None:
                kr = kraw_p.tile([P, C, 9], sd, tag="kr", name="kr")
            nc.scalar.activation(
                kr[:, lo:hi], krs[:, lo:hi],
                mybir.ActivationFunctionType.Copy)
            return kr

        def transpose_k_chunk(kr, kT, coc, cics=None):
            # PE-transpose each [co, ci] 128x128 tap block into kT[ci, co]
            for cic in (range(CCH) if cics is None else cics):
                for t in range(9):
                    ptr = tr_p.tile([P, P], sd, tag="tr", name="ptr")
                    nc.tensor.transpose(
                        ptr[:], kr[:, cic * P:(cic + 1) * P, t], ident
                        if mmd != F32R else ident.bitcast(F32))
                    nc.vector.tensor_copy(mm(kT[:, cic, coc, t, :]), ptr[:])

        def load_x_chunk(x_pad, b, c):
            dst = x_pad[:, c, 1:1 + H, 1:1 + W]
            src = x_d[b, c * P:(c + 1) * P]
            if mmd == F32R:
                # DMA to staging, then DVE pad-insert + fp32r rounding
                xs = xs_p.tile([P, H, W], F32, tag="xs", name="xs")
                nc.sync.dma_start(out=xs[:], in_=src)
                nc.vector.tensor_copy(dst.bitcast(F32R), xs[:])
            else:
                nc.sync.dma_start(out=dst, in_=src)

        def x_piece_dma(b, c, r0, r1):
            # bf16: HWDGE f32 DMA into the resident fp32 copy
            nc.sync.dma_start(
                out=xf[b % 2][:, c, r0:r1, :],
                in_=x_d[b, c * P:(c + 1) * P, r0:r1],
            )

        def x_piece_cast(x_pad, b, c, r0, r1, on_act=False):
            # pad-insert + cast to bf16; startup-critical pieces go on ACT,
            # whose queue is free while DVE drains its preamble + kT copies
            dst = x_pad[:, c, 1 + r0:1 + r1, 1:1 + W]
            src = xf[b % 2][:, c, r0:r1, :]
            if on_act:
                nc.scalar.activation(
                    dst, src, mybir.ActivationFunctionType.Copy)
            else:
                nc.vector.tensor_copy(dst, src)

        def load_x_piece(x_pad, b, c, r0, r1):
            x_piece_dma(b, c, r0, r1)
            x_piece_cast(x_pad, b, c, r0, r1)

        def emit_sweep(accs, out_cb, kT, src_pad, nt_list, coc, cic):
            """One cic's taps over a window of tiles; lets PE start on
            chunk-0 data while chunk-1 data still loads."""
            for i, nt in enumerate(nt_list):
                r0 = nt * TR
                for t in range(9):
                    dy, dx = t // 3, t % 3
                    nc.tensor.matmul(
                        accs[i][:],
                        mm(kT[:, cic, coc, t, :]),
                        mm(src_pad[:, cic, r0 + dy:r0 + dy + TR, dx:dx + W]),
                        start=(cic == 0 and t == 0),
                        stop=(cic == CCH - 1 and t == 8),
                    )
                if cic == CCH - 1:
                    out_cb(coc, nt, accs[i])

        def emit_conv(out_cb, kT, src_pad, resid_pad, nt_lo=0, nt_hi=NT,
                      cocs=None):
            for coc in (range(CCH) if cocs is None else cocs):
                for nt in range(nt_lo, nt_hi):
                    r0 = nt * TR
                    acc = acc_p.tile([P, TR, W], F32, tag="acc", name="acc")
                    n_mm = CCH * 9 + (1 if resid_pad is not None else 0)
                    i_mm = 0
                    for cic in range(CCH):
                        for t in range(9):
                            dy, dx = t // 3, t % 3
                            nc.tensor.matmul(
                                acc[:],
                                mm(kT[:, cic, coc, t, :]),
                                mm(src_pad[:, cic, r0 + dy:r0 + dy + TR,
                                           dx:dx + W]),
                                start=(i_mm == 0),
                                stop=(i_mm == n_mm - 1),
                            )
                            i_mm += 1
                    if resid_pad is not None:
                        nc.tensor.matmul(
                            acc[:],
                            ident if mmd != F32R else ident.bitcast(F32R),
                            mm(resid_pad[:, coc, 1 + r0:1 + r0 + TR, 1:1 + W]),
                            start=False,
                            stop=True,
                        )
                    out_cb(coc, nt, acc)

        P1, P2 = 16, 40  # x startup piece edges (conv1 nt 0..3 needs <= 33)
        for b in range(BPC):
            x_pad = xp[b % 2]

            def h_out(coc, nt, acc):
                r0 = nt * TR
                nc.scalar.activation(
                    mm(hp[:, coc, 1 + r0:1 + r0 + TR, 1:1 + W]), acc[:],
                    mybir.ActivationFunctionType.Relu)

            if sd == BF16 and b == 0:
                # Startup-ordered so PE never waits long: DMA dispatch order
                # is latency order (k1-coc0-cic0 first, then x pieces); the
                # ACT cast queue interleaves small k/x pieces so the first
                # sweep (coc=0, nt 0..3, cic=0 taps) can start while chunk-1
                # data still loads. Warmup dummies cover the PE ramp.
                # DMA-engine spin-up (~1us) via a tiny transfer dispatched
                # from the gpsimd queue so it doesn't delay the critical
                # k1 dispatch on the sync queue
                prewarm = xs_p.tile([P, 1], F32, tag="xs", name="prewarm")
                nc.gpsimd.dma_start(out=prewarm[:], in_=x_d[0, 0:P, 0, 0:1])
                zero_borders(x_pad)
                krs0 = kraw_p.tile([P, C, 9], F32, tag="krs", name="krs")
                src0 = k1_d[b, 0:P].rearrange("co ci kh kw -> co ci (kh kw)")
                nc.sync.dma_start(out=krs0[:, :P], in_=src0[:, :P])
                x_piece_dma(b, 0, 0, P1)
                x_piece_dma(b, 0, P1, P2)
                nc.sync.dma_start(out=krs0[:, P:], in_=src0[:, P:])
                x_piece_dma(b, 1, 0, P1)
                x_piece_dma(b, 1, P1, P2)
                krs1 = dma_k(k1_d, b, 1)
                # first weight cast on DVE (idle at startup; ~0.5us vs 1.25us
                # on ACT), freeing ACT for the x piece casts
                kr0 = kraw_p.tile([P, C, 9], sd, tag="kr", name="kr")
                nc.vector.tensor_copy(kr0[:, :P], krs0[:, :P])
                x_piece_cast(x_pad, b, 0, 0, P1, on_act=True)
                transpose_k_chunk(kr0, k1T, 0, cics=[0])
                x_piece_cast(x_pad, b, 0, P1, P2, on_act=True)
                cast_k(krs0, kr0, P, C)
                x_piece_cast(x_pad, b, 1, 0, P1, on_act=True)
                x_piece_cast(x_pad, b, 1, P1, P2, on_act=True)
                accs = [acc_p.tile([P, TR, W], F32, tag="acc", name="acc")
                        for _ in range(4)]
                emit_sweep(accs, h_out, k1T, x_pad, [0, 1, 2, 3], 0, 0)
                transpose_k_chunk(kr0, k1T, 0, cics=[1])
                emit_sweep(accs, h_out, k1T, x_pad, [0, 1, 2, 3], 0, 1)
                kr1 = cast_k(krs1)
                transpose_k_chunk(kr1, k1T, 1)
                for c in range(CCH):
                    load_x_piece(x_pad, b, c, P2, H)
                zero_borders(hp)
                emit_conv(h_out, k1T, x_pad, None, 0, 4, cocs=[1])
                emit_conv(h_out, k1T, x_pad, None, 4, NT)
            elif sd == BF16:
                # k1T, x_pad and xf were prefetched during the previous
                # sample's conv2
                emit_conv(h_out, k1T, x_pad, None)
            else:
                zero_borders(x_pad)
                for c in range(CCH):
                    kr = load_k_chunk(k1_d, b, c)
                    load_x_chunk(x_pad, b, c)
                    transpose_k_chunk(kr, k1T, c)
                if b == 0:
                    zero_borders(hp)
                emit_conv(h_out, k1T, x_pad, None)

            for c in range(CCH):
                kr = load_k_chunk(k2_d, b, c)
                transpose_k_chunk(kr, k2T, c)

            def y_out(coc, nt, acc):
                r0 = nt * TR
                ot = out_p.tile([P, TR, W], F32, tag="ot", name="ot")
                if sd == BF16:
                    # residual add on DVE from the exact fp32 x, relu on ACT
                    nc.vector.tensor_add(
                        ot[:], acc[:], xf[b % 2][:, coc, r0:r0 + TR, :])
                    nc.scalar.activation(
                        ot[:], ot[:], mybir.ActivationFunctionType.Relu)
                else:
                    nc.scalar.activation(
                        ot[:], acc[:], mybir.ActivationFunctionType.Relu)
                nc.sync.dma_start(
                    out=out_d[b, coc * P:(coc + 1) * P, r0:r0 + TR, :],
                    in_=ot[:],
                )

            if sd == BF16:
                emit_conv(y_out, k2T, hp, None, cocs=[0])
                if b + 1 < BPC:
                    # prefetch next sample's k1 + x under conv2's PE stream:
                    # k1T transposes land at the coc0->coc1 boundary, DMA and
                    # casts ride the idle DMA/DVE capacity.
                    nb = b + 1
                    x_pad_n = xp[nb % 2]
                    zero_borders(x_pad_n)
                    krn0 = load_k_chunk(k1_d, nb, 0)
                    for c in range(CCH):
                        x_piece_dma(nb, c, 0, H)
                        x_piece_cast(x_pad_n, nb, c, 0, H)
                    transpose_k_chunk(krn0, k1T, 0)
                    krn1 = load_k_chunk(k1_d, nb, 1)
                    transpose_k_chunk(krn1, k1T, 1)
                emit_conv(y_out, k2T, hp, None, cocs=[1])
            else:
                emit_conv(y_out, k2T, hp, x_pad)

    nc.compile()
    return nc


_NC_CACHE = {}


def _get_nc(mode):
    if mode not in _NC_CACHE:
        _NC_CACHE[mode] = build_nc(mode)
    return _NC_CACHE[mode]


def kernel(x, kernel1, kernel2, _trace=False, _mode="bf16"):
    x = np.ascontiguousarray(np.asarray(x, dtype=np.float32))
    kernel1 = np.ascontiguousarray(np.asarray(kernel1, dtype=np.float32))
    kernel2 = np.ascontiguousarray(np.asarray(kernel2, dtype=np.float32))
    nc = _get_nc(_mode)
    in_maps = [
        {
            "x": x[i * BPC:(i + 1) * BPC],
            "kernel1": kernel1[i * BPC:(i + 1) * BPC],
            "kernel2": kernel2[i * BPC:(i + 1) * BPC],
        }
        for i in range(N_CORES)
    ]
    last_err = None
    for attempt in range(3):
        try:
            res = run_bass_kernel_spmd(
                nc, in_maps, list(range(N_CORES)), trace=_trace)
            break
        except Exception as e:  # transient NRT device errors recover on retry
            last_err = e
            if "UNRECOVERABLE" not in str(e) and "UNAVAILABLE" not in str(e):
                raise
    else:
        raise last_err
    out = np.concatenate([res.results[i]["out"] for i in range(N_CORES)], axis=0)
    if _trace:
        return out, res
    return out



# revision 16
# speedup vs baseline: 1.0078x; 1.0078x over previous
"""Per-sample ResNet block (conv3x3 -> relu -> conv3x3 -> +x -> relu) on 8 trn2 cores.

Full inputs: x [16,256,64,64] f32, kernel1/kernel2 [16,256,256,3,3] f32.
Sharding: pure data parallelism, 2 samples per core.

Per-core bass/tile kernel:
  - x sample is stored in SBUF as two 128-channel chunks, zero-padded to 66x66
    so each conv tap (dy,dx) is a shifted AP slice.
  - conv = sum over (ci_chunk, tap) of matmul(lhsT=kT[ci,co], rhs=x_shift[ci,n])
    accumulated in PSUM over 18 matmuls per [128 co x 512 n] tile.
  - weights are DMA'd [co, ci*9] (contiguous) and transposed on the PE
    (out = in.T via identity) to get [ci, co] tiles.
  - residual add is one extra identity matmul into the same PSUM accumulation.
  - relu via ScalarE activation evacuates PSUM -> SBUF.

Modes (default bf16; measured on trn2, 8 cores):
  bf16: bf16 storage/matmuls, fp32 PSUM + exact fp32 residual add on DVE.
        ~287 us HW exec, rel err ~1.8e-3. FWL halves weight-load time, which
        fp32 dtypes cannot use, putting the LDW+MM cadence at the N=512
        streaming roofline.
  f32r: fp32 storage, matmuls in float32r (single-pass PE: 1 cyc/row at
        N>=256). ~326 us, rel err ~2e-4. Producers feeding fp32r matmuls must
        emit fp32r-tagged data (walrus birverifier rule), so x and the
        identity get DVE rounding copies and relu/weight copies write fp32r.
  f32:  plain fp32 matmuls (4 cyc/row). Slow reference fallback.
"""

import numpy as np
from contextlib import ExitStack

import concourse.bass as bass
import concourse.mybir as mybir
import concourse.tile as tile
from concourse import bacc
from concourse.bass_utils import run_bass_kernel_spmd
from concourse.masks import make_identity

N_CORES = 8
B_FULL = 16
BPC = B_FULL // N_CORES  # samples per core
C = 256
H = W = 64
HP = WP = 66  # padded
P = 128
CCH = C // P  # channel chunks: 2
NT = 8        # spatial tiles (rows of 8) per image: 64 rows / 8
TR = 8        # rows per spatial tile
F32 = mybir.dt.float32
F32R = mybir.dt.float32r
BF16 = mybir.dt.bfloat16


def build_nc(mode="f32r"):
    sd = BF16 if mode == "bf16" else F32          # storage dtype
    mmd = {"f32r": F32R, "bf16": BF16, "f32": F32}[mode]  # matmul dtype

    def mm(ap):
        # view a storage AP with the matmul dtype
        return ap.bitcast(mmd) if mmd != sd else ap

    nc = bacc.Bacc("TRN2", target_bir_lowering=False, debug=False)

    x_d = nc.dram_tensor("x", [BPC, C, H, W], F32, kind="ExternalInput")
    k1_d = nc.dram_tensor("kernel1", [BPC, C, C, 3, 3], F32, kind="ExternalInput")
    k2_d = nc.dram_tensor("kernel2", [BPC, C, C, 3, 3], F32, kind="ExternalInput")
    out_d = nc.dram_tensor("out", [BPC, C, H, W], F32, kind="ExternalOutput")

    with tile.TileContext(nc) as tc, ExitStack() as ctx:
        persist = ctx.enter_context(tc.tile_pool(name="persist", bufs=1))
        kraw_p = ctx.enter_context(tc.tile_pool(name="kraw", bufs=2))
        xs_p = ctx.enter_context(tc.tile_pool(name="xs", bufs=2))
        acc_p = ctx.enter_context(tc.tile_pool(name="acc", bufs=4, space="PSUM"))
        tr_p = ctx.enter_context(tc.tile_pool(name="tr", bufs=4, space="PSUM"))
        out_p = ctx.enter_context(tc.tile_pool(name="outs", bufs=4))

        ident = persist.tile([P, P], sd, tag="ident", name="ident")
        make_identity(nc, ident)
        if mmd == F32R:
            ident_r = persist.tile([P, P], F32, tag="ident_r", name="ident_r")
            nc.vector.tensor_copy(ident_r.bitcast(F32R), ident)
            ident = ident_r

        # PE p-state warmup: the tensor engine ramps to full clock only
        # after ~3us of continuous execution. Real weights arrive at ~11us
        # (DMA + cast latency); until then run dummy transposes so the
        # first real matmuls start at full speed instead of ~2x slower.
        if sd == BF16:
            wrhs = persist.tile([P, 512], sd, tag="wrhs", name="wrhs")
            nc.gpsimd.memset(wrhs, 0.0)

            def warm_mms(n):
                for _ in range(n):
                    warm = tr_p.tile([P, 512], F32, tag="tr", name="warm")
                    nc.tensor.matmul(warm[:], ident, wrhs[:], start=True,
                                     stop=True)

            warm_mms(17)

        # persistent padded images + transposed weights
        xp = [persist.tile([P, CCH, HP, WP], sd, tag=f"xp{i}", name=f"xp{i}")
              for i in range(2)]
        hp = persist.tile([P, CCH, HP, WP], sd, tag="hp", name="hp")
        # bf16 mode: keep an fp32 copy of x resident for the exact residual
        # add on DVE (replaces the identity matmul on PE)
        xf = None
        if sd == BF16:
            xf = [persist.tile([P, CCH, H, W], F32, tag=f"xf{i}", name=f"xf{i}")
                  for i in range(2)]
        k1T = persist.tile([P, CCH, CCH, 9, P], sd, tag="k1T", name="k1T")
        k2T = persist.tile([P, CCH, CCH, 9, P], sd, tag="k2T", name="k2T")

        # zero the 1-px borders of the padded tiles (via DVE copy from a zero
        # row: fp32r can't be memset directly, and the fp32r matmul requires
        # fp32r-tagged producers). Emitted lazily, just before first use.
        zrow = persist.tile([P, WP], sd, tag="zrow", name="zrow")
        nc.vector.memset(zrow, 0.0)

        def zero_borders(t):
            for c in range(CCH):
                if sd == BF16:
                    # gpsimd is idle and has no framework preamble backlog
                    nc.gpsimd.memset(t[:, c, 0, :], 0.0)
                    nc.gpsimd.memset(t[:, c, HP - 1, :], 0.0)
                    nc.gpsimd.memset(t[:, c, 1:HP - 1, 0], 0.0)
                    nc.gpsimd.memset(t[:, c, 1:HP - 1, WP - 1], 0.0)
                else:
                    nc.vector.tensor_copy(mm(t[:, c, 0, :]), zrow[:, :WP])
                    nc.vector.tensor_copy(mm(t[:, c, HP - 1, :]), zrow[:, :WP])
                    nc.vector.tensor_copy(
                        mm(t[:, c, 1:HP - 1, 0]), zrow[:, :HP - 2])
                    nc.vector.tensor_copy(
                        mm(t[:, c, 1:HP - 1, WP - 1]), zrow[:, :HP - 2])

        def load_k_chunk(k_d, b, coc):
            kr = kraw_p.tile([P, C, 9], sd, tag="kr", name="kr")
            src = k_d[b, coc * P:(coc + 1) * P].rearrange(
                "co ci kh kw -> co ci (kh kw)")
            if sd == F32:
                nc.sync.dma_start(out=kr[:], in_=src)
            else:
                # HWDGE f32 DMA to staging, then cast to bf16 on ACT (keeps
                # the startup-critical DVE queue free; SWDGE cast-DMA stalls
                # on Q7 descriptor emission)
                krs = kraw_p.tile([P, C, 9], F32, tag="krs", name="krs")
                nc.sync.dma_start(out=krs[:], in_=src)
                nc.scalar.activation(
                    kr[:], krs[:], mybir.ActivationFunctionType.Copy)
            return kr

        def dma_k(k_d, b, coc, split=False):
            # f32 staging DMA only; cast emitted separately (startup latency)
            krs = kraw_p.tile([P, C, 9], F32, tag="krs", name="krs")
            src = k_d[b, coc * P:(coc + 1) * P].rearrange(
                "co ci kh kw -> co ci (kh kw)")
            if split:
                nc.sync.dma_start(out=krs[:, :P], in_=src[:, :P])
                nc.sync.dma_start(out=krs[:, P:], in_=src[:, P:])
            else:
                nc.sync.dma_start(out=krs[:], in_=src)
            return krs

        def cast_k(krs, kr=None, lo=0, hi=C):
            if kr is None:
                kr = kraw_p.tile([P, C, 9], sd, tag="kr", name="kr")
            nc.scalar.activation(
                kr[:, lo:hi], krs[:, lo:hi],
                mybir.ActivationFunctionType.Copy)
            return kr

        def transpose_k_chunk(kr, kT, coc, cics=None):
            # PE-transpose each [co, ci] 128x128 tap block into kT[ci, co]
            for cic in (range(CCH) if cics is None else cics):
                for t in range(9):
                    ptr = tr_p.tile([P, P], sd, tag="tr", name="ptr")
                    nc.tensor.transpose(
                        ptr[:], kr[:, cic * P:(cic + 1) * P, t], ident
                        if mmd != F32R else ident.bitcast(F32))
                    nc.vector.tensor_copy(mm(kT[:, cic, coc, t, :]), ptr[:])

        def load_x_chunk(x_pad, b, c):
            dst = x_pad[:, c, 1:1 + H, 1:1 + W]
            src = x_d[b, c * P:(c + 1) * P]
            if mmd == F32R:
                # DMA to staging, then DVE pad-insert + fp32r rounding
                xs = xs_p.tile([P, H, W], F32, tag="xs", name="xs")
                nc.sync.dma_start(out=xs[:], in_=src)
                nc.vector.tensor_copy(dst.bitcast(F32R), xs[:])
            else:
                nc.sync.dma_start(out=dst, in_=src)

        def x_piece_dma(b, c, r0, r1):
            # bf16: HWDGE f32 DMA into the resident fp32 copy
            nc.sync.dma_start(
                out=xf[b % 2][:, c, r0:r1, :],
                in_=x_d[b, c * P:(c + 1) * P, r0:r1],
            )

        def x_piece_cast(x_pad, b, c, r0, r1, on_act=False):
            # pad-insert + cast to bf16; startup-critical pieces go on ACT,
            # whose queue is free while DVE drains its preamble + kT copies
            dst = x_pad[:, c, 1 + r0:1 + r1, 1:1 + W]
            src = xf[b % 2][:, c, r0:r1, :]
            if on_act:
                nc.scalar.activation(
                    dst, src, mybir.ActivationFunctionType.Copy)
            else:
                nc.vector.tensor_copy(dst, src)

        def load_x_piece(x_pad, b, c, r0, r1):
            x_piece_dma(b, c, r0, r1)
            x_piece_cast(x_pad, b, c, r0, r1)

        def emit_sweep(accs, out_cb, kT, src_pad, nt_list, coc, cic):
            """One cic's taps over a window of tiles; lets PE start on
            chunk-0 data while chunk-1 data still loads."""
            for i, nt in enumerate(nt_list):
                r0 = nt * TR
                for t in range(9):
                    dy, dx = t // 3, t % 3
                    nc.tensor.matmul(
                        accs[i][:],
                        mm(kT[:, cic, coc, t, :]),
                        mm(src_pad[:, cic, r0 + dy:r0 + dy + TR, dx:dx + W]),
                        start=(cic == 0 and t == 0),
                        stop=(cic == CCH - 1 and t == 8),
                    )
                if cic == CCH - 1:
                    out_cb(coc, nt, accs[i])

        def emit_conv(out_cb, kT, src_pad, resid_pad, nt_lo=0, nt_hi=NT,
                      cocs=None):
            for coc in (range(CCH) if cocs is None else cocs):
                for nt in range(nt_lo, nt_hi):
                    r0 = nt * TR
                    acc = acc_p.tile([P, TR, W], F32, tag="acc", name="acc")
                    n_mm = CCH * 9 + (1 if resid_pad is not None else 0)
                    i_mm = 0
                    for cic in range(CCH):
                        for t in range(9):
                            dy, dx = t // 3, t % 3
                            nc.tensor.matmul(
                                acc[:],
                                mm(kT[:, cic, coc, t, :]),
                                mm(src_pad[:, cic, r0 + dy:r0 + dy + TR,
                                           dx:dx + W]),
                                start=(i_mm == 0),
                                stop=(i_mm == n_mm - 1),
                            )
                            i_mm += 1
                    if resid_pad is not None:
                        nc.tensor.matmul(
                            acc[:],
                            ident if mmd != F32R else ident.bitcast(F32R),
                            mm(resid_pad[:, coc, 1 + r0:1 + r0 + TR, 1:1 + W]),
                            start=False,
                            stop=True,
                        )
                    out_cb(coc, nt, acc)

        P1, P2 = 16, 40  # x startup piece edges (conv1 nt 0..3 needs <= 33)
        for b in range(BPC):
            x_pad = xp[b % 2]

            def h_out(coc, nt, acc):
                r0 = nt * TR
                nc.scalar.activation(
                    mm(hp[:, coc, 1 + r0:1 + r0 + TR, 1:1 + W]), acc[:],
                    mybir.ActivationFunctionType.Relu)

            if sd == BF16 and b == 0:
                # Startup-ordered so PE never waits long: DMA dispatch order
                # is latency order (k1-coc0-cic0 first, then x pieces); the
                # ACT cast queue interleaves small k/x pieces so the first
                # sweep (coc=0, nt 0..3, cic=0 taps) can start while chunk-1
                # data still loads. Warmup dummies cover the PE ramp.
                zero_borders(x_pad)
                krs0 = kraw_p.tile([P, C, 9], F32, tag="krs", name="krs")
                src0 = k1_d[b, 0:P].rearrange("co ci kh kw -> co ci (kh kw)")
                nc.sync.dma_start(out=krs0[:, :P], in_=src0[:, :P])
                x_piece_dma(b, 0, 0, P1)
                x_piece_dma(b, 0, P1, P2)
                nc.sync.dma_start(out=krs0[:, P:], in_=src0[:, P:])
                x_piece_dma(b, 1, 0, P1)
                x_piece_dma(b, 1, P1, P2)
                krs1 = dma_k(k1_d, b, 1)
                kr0 = cast_k(krs0, None, 0, P)
                x_piece_cast(x_pad, b, 0, 0, P1, on_act=True)
                transpose_k_chunk(kr0, k1T, 0, cics=[0])
                x_piece_cast(x_pad, b, 0, P1, P2, on_act=True)
                cast_k(krs0, kr0, P, C)
                x_piece_cast(x_pad, b, 1, 0, P1, on_act=True)
                x_piece_cast(x_pad, b, 1, P1, P2, on_act=True)
                warm_mms(3)  # bridge PE over the x-piece-cast wait
                accs = [acc_p.tile([P, TR, W], F32, tag="acc", name="acc")
                        for _ in range(4)]
                emit_sweep(accs, h_out, k1T, x_pad, [0, 1, 2, 3], 0, 0)
                transpose_k_chunk(kr0, k1T, 0, cics=[1])
                emit_sweep(accs, h_out, k1T, x_pad, [0, 1, 2, 3], 0, 1)
                kr1 = cast_k(krs1)
                transpose_k_chunk(kr1, k1T, 1)
                for c in range(CCH):
                    load_x_piece(x_pad, b, c, P2, H)
                zero_borders(hp)
                emit_conv(h_out, k1T, x_pad, None, 0, 4, cocs=[1])
                emit_conv(h_out, k1T, x_pad, None, 4, NT)
            elif sd == BF16:
                # k1T, x_pad and xf were prefetched during the previous
                # sample's conv2
                emit_conv(h_out, k1T, x_pad, None)
            else:
                zero_borders(x_pad)
                for c in range(CCH):
                    kr = load_k_chunk(k1_d, b, c)
                    load_x_chunk(x_pad, b, c)
                    transpose_k_chunk(kr, k1T, c)
                if b == 0:
                    zero_borders(hp)
                emit_conv(h_out, k1T, x_pad, None)

            for c in range(CCH):
                kr = load_k_chunk(k2_d, b, c)
                transpose_k_chunk(kr, k2T, c)

            def y_out(coc, nt, acc):
                r0 = nt * TR
                ot = out_p.tile([P, TR, W], F32, tag="ot", name="ot")
                if sd == BF16:
                    # residual add on DVE from the exact fp32 x, relu on ACT
                    nc.vector.tensor_add(
                        ot[:], acc[:], xf[b % 2][:, coc, r0:r0 + TR, :])
                    nc.scalar.activation(
                        ot[:], ot[:], mybir.ActivationFunctionType.Relu)
                else:
                    nc.scalar.activation(
                        ot[:], acc[:], mybir.ActivationFunctionType.Relu)
                nc.sync.dma_start(
                    out=out_d[b, coc * P:(coc + 1) * P, r0:r0 + TR, :],
                    in_=ot[:],
                )

            if sd == BF16:
                emit_conv(y_out, k2T, hp, None, cocs=[0])
                if b + 1 < BPC:
                    # prefetch next sample's k1 + x under conv2's PE stream:
                    # k1T transposes land at the coc0->coc1 boundary, DMA and
                    # casts ride the idle DMA/DVE capacity.
                    nb = b + 1
                    x_pad_n = xp[nb % 2]
                    zero_borders(x_pad_n)
                    krn0 = load_k_chunk(k1_d, nb, 0)
                    for c in range(CCH):
                        x_piece_dma(nb, c, 0, H)
                        x_piece_cast(x_pad_n, nb, c, 0, H)
                    transpose_k_chunk(krn0, k1T, 0)
                    krn1 = load_k_chunk(k1_d, nb, 1)
                    transpose_k_chunk(krn1, k1T, 1)
                emit_conv(y_out, k2T, hp, None, cocs=[1])
            else:
                emit_conv(y_out, k2T, hp, x_pad)

    nc.compile()
    return nc


_NC_CACHE = {}


def _get_nc(mode):
    if mode not in _NC_CACHE:
        _NC_CACHE[mode] = build_nc(mode)
    return _NC_CACHE[mode]


def kernel(x, kernel1, kernel2, _trace=False, _mode="bf16"):
    x = np.ascontiguousarray(np.asarray(x, dtype=np.float32))
    kernel1 = np.ascontiguousarray(np.asarray(kernel1, dtype=np.float32))
    kernel2 = np.ascontiguousarray(np.asarray(kernel2, dtype=np.float32))
    nc = _get_nc(_mode)
    in_maps = [
        {
            "x": x[i * BPC:(i + 1) * BPC],
            "kernel1": kernel1[i * BPC:(i + 1) * BPC],
            "kernel2": kernel2[i * BPC:(i + 1) * BPC],
        }
        for i in range(N_CORES)
    ]
    last_err = None
    for attempt in range(3):
        try:
            res = run_bass_kernel_spmd(
                nc, in_maps, list(range(N_CORES)), trace=_trace)
            break
        except Exception as e:  # transient NRT device errors recover on retry
            last_err = e
            if "UNRECOVERABLE" not in str(e) and "UNAVAILABLE" not in str(e):
                raise
    else:
        raise last_err
    out = np.concatenate([res.results[i]["out"] for i in range(N_CORES)], axis=0)
    if _trace:
        return out, res
    return out

